# revision 1
# baseline (speedup 1.0000x reference)
"""Trainium2 Bass kernel for nn_BlockWithCache (Music-Transformer block w/ rel-pos).

Sharding (8 NeuronCores, uniform SPMD program; per-core differences live in the
input data only):
  - core c: batch element b = c//2, tensor-parallel half = c%2.
  - Attention: TP over heads — each core computes its 8 of 16 heads for the
    full 1024-token sequence (weight column slices supplied by the host).
  - Wproj row-slices produce partial attention outputs; a pairwise
    ReduceScatter(add) both completes the sum and splits tokens in half.
  - From the residual on: token-split — each core owns 512 tokens through
    LN2 + FFN (full 4*D hidden) and writes a disjoint output half.

Key tricks:
  - fp32r matmuls (full PE rate at free-dim>=256, fp32 operands/accum).
  - Music-Transformer skew: QEr rows round-trip through a DRAM buffer written
    with row stride 128*(qc+2) and read back with row stride 128*(qc+2)-1,
    which realigns QEr[q, 1023-q+c] to [q, c]; the 128-wide pad region holds
    -1e9 so the causal mask comes back for free (exp -> 0).
  - Softmax without max-subtraction (logits are small; fp32 exp is safe),
    denominator via the ACT engine's fused accum_out.
  - attT via PE transpose (bf16); att@V and denominators in bf16.
"""

import os
import sys

os.environ.setdefault("MYCRO_LOCAL_CACHE", "1")
if "/opt/trn_rl_repo" not in sys.path:
    sys.path.insert(0, "/opt/trn_rl_repo")

import numpy as np

B, L, D, H = 4, 1024, 1024, 16
HS = D // H          # 64
P = 128
TC = L // P          # 8 token chunks
DCH = D // P         # 8 feature chunks
NHC = H // 2         # 8 heads per core
FD = 4 * D           # 4096
FC = FD // P         # 32
TMY = L // 2         # 512 tokens owned after RS
T2 = TMY // P        # 4
EPS = 1e-5
SCALE = 1.0 / 8.0    # 1/sqrt(HS)
NEG = -1.0e9

_PROGRAM_CACHE = {}


def _build_program(flags, no_rs=False):
    import concourse.mybir as mybir
    import concourse.tile as tile
    from concourse import bacc
    from concourse.masks import make_identity

    (aff1, aff2, use_bq, use_bk, use_bv, use_bproj, use_bfc, use_bfc2) = flags

    f32 = mybir.dt.float32
    f32r = mybir.dt.float32r
    bf16 = mybir.dt.bfloat16
    AF = mybir.ActivationFunctionType
    ALU = mybir.AluOpType
    AX = mybir.AxisListType

    nc = bacc.Bacc("TRN2", target_bir_lowering=False, debug=False, num_devices=8)

    x_in = nc.declare_dram_parameter("x", [L, D], f32, isOutput=False)
    xmy_in = nc.declare_dram_parameter("x_my", [TMY, D], f32, isOutput=False)
    wq_in = nc.declare_dram_parameter("wq", [D, NHC * HS], f32, isOutput=False)
    wk_in = nc.declare_dram_parameter("wk", [D, NHC * HS], f32, isOutput=False)
    wv_in = nc.declare_dram_parameter("wv", [D, NHC * HS], f32, isOutput=False)
    wproj_in = nc.declare_dram_parameter("wproj", [NHC * HS, D], f32, isOutput=False)
    ert2_in = nc.declare_dram_parameter("ert2", [P, L], f32, isOutput=False)
    wfc_in = nc.declare_dram_parameter("wfc", [D, FD], f32, isOutput=False)
    wfc2_in = nc.declare_dram_parameter("wfc2", [FD, D], f32, isOutput=False)
    # Always-declared small params (cheap; used only when flags set)
    ln1a_in = nc.declare_dram_parameter("ln1a", [D], f32, isOutput=False)
    ln1b_in = nc.declare_dram_parameter("ln1b", [D], f32, isOutput=False)
    ln2a_in = nc.declare_dram_parameter("ln2a", [D], f32, isOutput=False)
    ln2b_in = nc.declare_dram_parameter("ln2b", [D], f32, isOutput=False)
    bq_in = nc.declare_dram_parameter("bq", [P, 4], f32, isOutput=False)
    bk_in = nc.declare_dram_parameter("bk", [P, 4], f32, isOutput=False)
    bv_in = nc.declare_dram_parameter("bv", [NHC * HS], f32, isOutput=False)
    bproj_in = nc.declare_dram_parameter("bproj", [D], f32, isOutput=False)
    bfc_in = nc.declare_dram_parameter("bfc", [P, FC], f32, isOutput=False)
    bfc2_in = nc.declare_dram_parameter("bfc2", [D], f32, isOutput=False)

    out_dram = nc.declare_dram_parameter("out_my", [TMY, D], f32, isOutput=True)

    def layernorm(tc, nc, pools, xs, hs, nchunks, aff, wbc, bbc, eps_ap):
        """Per-chunk two-pass LN so chunk t's output is ready without waiting
        on later chunks (keeps the downstream transposes/matmuls flowing)."""
        small, scratch = pools
        for t in range(nchunks):
            st = small.tile([P, 8], f32, tag="ln_st")
            # st cols: 0 sum, 1 sumsq, 2 mu, 3 mu^2, 4 var, 5 std, 6 rstd, 7 mur
            nc.vector.reduce_sum(st[:, 0:1], xs[t][:], axis=AX.X)
            sq = scratch.tile([P, D], f32, tag="ln_sq")
            nc.scalar.activation(sq[:], xs[t][:], AF.Square, accum_out=st[:, 1:2])
            nc.vector.tensor_scalar_mul(st[:, 2:3], st[:, 0:1], 1.0 / D)
            nc.vector.tensor_tensor(st[:, 3:4], st[:, 2:3], st[:, 2:3], op=ALU.mult)
            nc.vector.tensor_scalar(
                st[:, 4:5], st[:, 1:2], 1.0 / D, st[:, 3:4],
                op0=ALU.mult, op1=ALU.subtract,
            )
            nc.scalar.activation(st[:, 5:6], st[:, 4:5], AF.Sqrt, bias=eps_ap)
            nc.vector.reciprocal(st[:, 6:7], st[:, 5:6])
            nc.vector.tensor_tensor(st[:, 7:8], st[:, 2:3], st[:, 6:7], op=ALU.mult)
            nc.vector.tensor_scalar(
                hs[t][:],
                xs[t][:],
                st[:, 6:7],
                st[:, 7:8],
                op0=ALU.mult,
                op1=ALU.subtract,
            )
            if aff:
                nc.vector.tensor_tensor(hs[t][:], hs[t][:], wbc[:], op=ALU.mult)
                nc.vector.tensor_tensor(hs[t][:], hs[t][:], bbc[:], op=ALU.add)

    with tile.TileContext(nc) as tc:
        import contextlib

        with contextlib.ExitStack() as es:
            cst = es.enter_context(tc.tile_pool(name="cst", bufs=1))
            small = es.enter_context(tc.tile_pool(name="small", bufs=2))
            dram = es.enter_context(tc.tile_pool(name="dram", bufs=1, space="DRAM"))

            h2Tp = es.enter_context(tc.tile_pool(name="h2Tp", bufs=1))

            eps_t = cst.tile([P, 1], f32)
            nc.vector.memset(eps_t[:], EPS)
            warm = cst.tile([P, 2], f32)
            nc.vector.memset(warm[:], 1.0)
            for fn in (AF.Square, AF.Sqrt, AF.Exp, AF.Gelu, AF.Copy):
                nc.scalar.activation(warm[:, 1:2], warm[:, 0:1], fn)
            id32 = cst.tile([P, P], f32)
            make_identity(nc, id32)
            id16 = cst.tile([P, P], bf16)
            make_identity(nc, id16)
            ert2 = cst.tile([P, L], f32r)
            nc.sync.dma_start(ert2[:], ert2_in[:].bitcast(f32r))

            ln1w_bc = ln1b_bc = ln2w_bc = ln2b_bc = None
            if aff1:
                row = cst.tile([1, D], f32, tag="lnrow1a")
                nc.sync.dma_start(row[:], ln1a_in[None, :])
                ln1w_bc = cst.tile([P, D], f32)
                nc.gpsimd.partition_broadcast(ln1w_bc[:], row[:])
                row2 = cst.tile([1, D], f32, tag="lnrow1b")
                nc.sync.dma_start(row2[:], ln1b_in[None, :])
                ln1b_bc = cst.tile([P, D], f32)
                nc.gpsimd.partition_broadcast(ln1b_bc[:], row2[:])
            if aff2:
                row = cst.tile([1, D], f32, tag="lnrow2a")
                nc.sync.dma_start(row[:], ln2a_in[None, :])
                ln2w_bc = cst.tile([P, D], f32)
                nc.gpsimd.partition_broadcast(ln2w_bc[:], row[:])
                row2 = cst.tile([1, D], f32, tag="lnrow2b")
                nc.sync.dma_start(row2[:], ln2b_in[None, :])
                ln2b_bc = cst.tile([P, D], f32)
                nc.gpsimd.partition_broadcast(ln2b_bc[:], row2[:])
            bq_sb = bk_sb = None
            if use_bq:
                bq_sb = cst.tile([P, 4], f32)
                nc.sync.dma_start(bq_sb[:], bq_in[:])
            if use_bk:
                bk_sb = cst.tile([P, 4], f32)
                nc.sync.dma_start(bk_sb[:], bk_in[:])
            bv_bc = None
            if use_bv:
                row = cst.tile([1, NHC * HS], f32, tag="bvrow")
                nc.sync.dma_start(row[:], bv_in[None, :])
                bv_bc = cst.tile([P, NHC * HS], f32)
                nc.gpsimd.partition_broadcast(bv_bc[:], row[:])
            bproj_bc = None
            if use_bproj:
                row = cst.tile([1, D], f32, tag="bprow")
                nc.sync.dma_start(row[:], bproj_in[None, :])
                bproj_bc = cst.tile([P, D], f32)
                nc.gpsimd.partition_broadcast(bproj_bc[:], row[:])
            bfc_sb = None
            if use_bfc:
                bfc_sb = cst.tile([P, FC], f32)
                nc.sync.dma_start(bfc_sb[:], bfc_in[:])
            bfc2_bc = None
            if use_bfc2:
                row = cst.tile([1, D], f32, tag="b2row")
                nc.sync.dma_start(row[:], bfc2_in[None, :])
                bfc2_bc = cst.tile([P, D], f32)
                nc.gpsimd.partition_broadcast(bfc2_bc[:], row[:])

            # Skew DRAM buffers: per q-chunk, 2 slots, pad cols = NEG

            cc_in = [dram.tile([L, 512], f32, name=f"cc_in{n}") for n in range(2)]
            cc_out = [dram.tile([TMY, 512], f32, name=f"cc_out{n}") for n in range(2)]

            # ---------------- persistent activation tiles ----------------
            ysb_pool = tc.alloc_tile_pool(name="ysb", bufs=1)
            ysb = ysb_pool.tile([P, 4, L], f32r)
            negpad = cst.tile([P, P], bf16)
            nc.vector.memset(negpad[:], NEG)
            skewbufs = []
            for qc in range(TC):
                srow = P * (qc + 2)
                wm = P * (qc + 1)
                slots = []
                for s in range(2):
                    d1 = dram.tile([P * srow], bf16, name=f"skew_{qc}_{s}")
                    wv_full = d1[:].rearrange("(r c) -> r c", c=srow)
                    nc.sync.dma_start(wv_full[:, wm:], negpad[:])
                    slots.append(d1)
                skewbufs.append(slots)

            qkv_pool = tc.alloc_tile_pool(name="qkv", bufs=1)
            qt_sb = [qkv_pool.tile([P, L], f32r, name=f"qt{p}") for p in range(4)]
            kt_sb = [qkv_pool.tile([P, L], f32r, name=f"kt{p}") for p in range(4)]
            v_sb = [qkv_pool.tile([P, NHC * HS], bf16, name=f"v{t}") for t in range(TC)]

            xph = tc.alloc_tile_pool(name="xp", bufs=1)
            xs_tiles = [xph.tile([P, D], f32, name=f"x{t}") for t in range(TC)]
            for t in range(TC):
                nc.sync.dma_start(xs_tiles[t][:], x_in[t * P : (t + 1) * P, :])

            # ---------------- LN1 + transpose + QKV ----------------
            with tc.tile_pool(name="hT", bufs=1) as hTp:
                hT = [hTp.tile([P, L], f32r, name=f"hT{d}") for d in range(DCH)]
                with tc.tile_pool(name="xh", bufs=1) as xh, tc.tile_pool(
                    name="lnscr", bufs=2
                ) as lnscr:
                    xs = xs_tiles
                    hs = [xh.tile([P, D], f32, name=f"h{t}") for t in range(TC)]
                    layernorm(
                        tc, nc, (small, lnscr), xs, hs, TC, aff1, ln1w_bc, ln1b_bc,
                        eps_t[:],
                    )
                    with tc.tile_pool(name="htps", bufs=4, space="PSUM") as htps:
                        for t in range(TC):
                            for d in range(DCH):
                                tp = htps.tile([P, P], f32, tag="htp")
                                nc.tensor.transpose(
                                    tp[:], hs[t][:, d * P : (d + 1) * P], id32[:]
                                )
                                nc.any.tensor_copy(
                                    hT[d][:, t * P : (t + 1) * P], tp[:]
                                )

                # QKV projections (h freed; hT alive)
                with tc.tile_pool(name="wqkv", bufs=1) as wp, tc.tile_pool(
                    name="qkvps", bufs=4, space="PSUM"
                ) as qps:
                    wq_sb = [wp.tile([P, NHC * HS], f32r, name=f"wq{d}") for d in range(DCH)]
                    wk_sb = [wp.tile([P, NHC * HS], f32r, name=f"wk{d}") for d in range(DCH)]
                    wv_sb = [wp.tile([P, NHC * HS], f32r, name=f"wv{d}") for d in range(DCH)]
                    for d in range(DCH):
                        nc.sync.dma_start(wq_sb[d][:], wq_in[d * P : (d + 1) * P, :].bitcast(f32r))
                        nc.sync.dma_start(wk_sb[d][:], wk_in[d * P : (d + 1) * P, :].bitcast(f32r))
                        nc.sync.dma_start(wv_sb[d][:], wv_in[d * P : (d + 1) * P, :].bitcast(f32r))
                    # Q^T and K^T: out [128(2 heads), tokens]
                    for p in range(4):
                        for n in range(2):
                            ps = qps.tile([P, 512], f32, tag="qkvp")
                            for d in range(DCH):
                                nc.tensor.matmul(
                                    ps[:],
                                    wq_sb[d][:, p * P : (p + 1) * P],
                                    hT[d][:, n * 512 : (n + 1) * 512],
                                    start=(d == 0),
                                    stop=(d == DCH - 1),
                                )
                            if use_bq:
                                nc.scalar.activation(
                                    qt_sb[p][:, n * 512 : (n + 1) * 512],
                                    ps[:],
                                    AF.Copy,
                                    scale=SCALE,
                                )
                                nc.vector.tensor_scalar_add(
                                    qt_sb[p][:, n * 512 : (n + 1) * 512],
                                    qt_sb[p][:, n * 512 : (n + 1) * 512],
                                    bq_sb[:, p : p + 1],
                                )
                            else:
                                nc.scalar.activation(
                                    qt_sb[p][:, n * 512 : (n + 1) * 512],
                                    ps[:],
                                    AF.Copy,
                                    scale=SCALE,
                                )
                        for n in range(2):
                            ps = qps.tile([P, 512], f32, tag="qkvp")
                            for d in range(DCH):
                                nc.tensor.matmul(
                                    ps[:],
                                    wk_sb[d][:, p * P : (p + 1) * P],
                                    hT[d][:, n * 512 : (n + 1) * 512],
                                    start=(d == 0),
                                    stop=(d == DCH - 1),
                                )
                            nc.scalar.activation(
                                kt_sb[p][:, n * 512 : (n + 1) * 512], ps[:], AF.Copy
                            )
                            if use_bk:
                                nc.vector.tensor_scalar_add(
                                    kt_sb[p][:, n * 512 : (n + 1) * 512],
                                    kt_sb[p][:, n * 512 : (n + 1) * 512],
                                    bk_sb[:, p : p + 1],
                                )
                    # V: out [tokens, 512 hs-cols]
                    for t in range(TC):
                        ps = qps.tile([P, 512], f32, tag="qkvp")
                        for d in range(DCH):
                            nc.tensor.matmul(
                                ps[:],
                                hT[d][:, t * P : (t + 1) * P],
                                wv_sb[d][:],
                                start=(d == 0),
                                stop=(d == DCH - 1),
                            )
                        if use_bv:
                            nc.vector.tensor_tensor(
                                ps[:], ps[:], bv_bc[:], op=ALU.add
                            )
                        nc.scalar.activation(v_sb[t][:], ps[:], AF.Copy)

            xph.release()

            # ---------------- attention ----------------
            with contextlib.ExitStack() as att_es:
                expp = att_es.enter_context(tc.tile_pool(name="expp", bufs=6))
                srelp = att_es.enter_context(tc.tile_pool(name="srelp", bufs=4))
                rsbp = att_es.enter_context(tc.tile_pool(name="rsbp", bufs=4))
                attTp = att_es.enter_context(tc.tile_pool(name="attTp", bufs=3))
                dnp = att_es.enter_context(tc.tile_pool(name="dnp", bufs=4))
                sps = att_es.enter_context(tc.tile_pool(name="sps", bufs=3, space="PSUM"))
                rps = att_es.enter_context(tc.tile_pool(name="rps", bufs=1, space="PSUM"))
                tps = att_es.enter_context(tc.tile_pool(name="tps", bufs=2, space="PSUM"))
                yps = att_es.enter_context(tc.tile_pool(name="yps", bufs=1, space="PSUM"))

                def emit_rphase(h):
                    """R = Q Er^T -> DRAM skew write -> skewed read (Srel)."""
                    p, hodd = divmod(h, 2)
                    off = hodd * 64
                    srels = []
                    for qc in range(TC):
                        wp_ = P * (qc + 1)
                        m0 = 896 - P * qc
                        srow = P * (qc + 2)
                        nsub = (wp_ + 511) // 512
                        lhsq = qt_sb[p][off : off + 64, qc * P : (qc + 1) * P]
                        d1 = skewbufs[qc][h % 2]
                        wview = d1[:].rearrange("(r c) -> r c", c=srow)
                        rview = d1[127 : 127 + P * (srow - 1)].rearrange(
                            "(r c) -> r c", c=srow - 1
                        )
                        rsb = rsbp.tile([P, wp_], bf16, tag="rsb")
                        for s in range(nsub):
                            w = min(512, wp_ - s * 512)
                            rp = rps.tile([P, 512], f32, tag="rp")
                            nc.tensor.matmul(
                                rp[:, :w],
                                lhsq,
                                ert2[off : off + 64, m0 + s * 512 : m0 + s * 512 + w],
                                start=True,
                                stop=True,
                            )
                            nc.vector.tensor_copy(
                                rsb[:, s * 512 : s * 512 + w], rp[:, :w]
                            )
                        nc.sync.dma_start(wview[:, :wp_], rsb[:])
                        srel = srelp.tile([P, wp_], bf16, tag=f"srel{qc}")
                        nc.sync.dma_start(srel[:], rview[:, :wp_])
                        srels.append(srel)
                    return srels

                wfc_pre = {}
                srel_pending = {0: [emit_rphase(0), emit_rphase(1)]}
                for pr in range(4):
                    h0, h1 = 2 * pr, 2 * pr + 1
                    if pr + 1 < 4:
                        srel_pending[pr + 1] = [
                            emit_rphase(2 * pr + 2),
                            emit_rphase(2 * pr + 3),
                        ]
                    srels2 = srel_pending.pop(pr)
                    attT2 = [
                        attTp.tile([P, TC, L], bf16, tag="attT", name=f"attT_{pr}_{i}")
                        for i in range(2)
                    ]
                    dn = dnp.tile([P, 2, TC, 2], f32, tag="dn")
                    dns = dnp.tile([P, 2, TC], f32, tag="dns")
                    rc = dnp.tile([P, 2, TC], f32, tag="rc")
                    for qc in range(TC):
                        wp_ = P * (qc + 1)     # W' = causal width
                        nsub = (wp_ + 511) // 512
                        lhsq2 = [
                            qt_sb[pr][0:64, qc * P : (qc + 1) * P],
                            qt_sb[pr][64:128, qc * P : (qc + 1) * P],
                        ]
                        exp2 = [
                            expp.tile([P, wp_], bf16, tag="exp", name=f"ex_{pr}_{qc}_{i}")
                            for i in range(2)
                        ]
                        for s in range(nsub):
                            w = min(512, wp_ - s * 512)
                            sl = slice(s * 512, s * 512 + w)
                            sp2 = [
                                sps.tile([P, 512], f32, tag="sp", name=f"sp_{qc}_{s}_{i}")
                                for i in range(2)
                            ]
                            # the two heads' QK matmuls use disjoint PE row
                            # groups (K rows 0-63 vs 64-127) -> run concurrent
                            for i in range(2):
                                nc.tensor.matmul(
                                    sp2[i][:, :w],
                                    lhsq2[i],
                                    kt_sb[pr][64 * i : 64 * i + 64, sl],
                                    start=True,
                                    stop=False,
                                )
                            # += Srel (with -1e9 causal pad) via identity matmul
                            for i in range(2):
                                nc.tensor.matmul(
                                    sp2[i][:, :w],
                                    id16[:],
                                    srels2[i][qc][:, sl],
                                    start=False,
                                    stop=True,
                                )
                            for i in range(2):
                                nc.scalar.activation(
                                    exp2[i][:, sl], sp2[i][:, :w], AF.Exp,
                                    accum_out=dn[:, i, qc, s : s + 1],
                                )
                        for i in range(2):
                            if nsub == 2:
                                nc.vector.tensor_tensor(
                                    dns[:, i, qc : qc + 1],
                                    dn[:, i, qc, 0:1],
                                    dn[:, i, qc, 1:2],
                                    op=ALU.add,
                                )
                            else:
                                nc.vector.tensor_copy(
                                    dns[:, i, qc : qc + 1], dn[:, i, qc, 0:1]
                                )
                            nc.vector.reciprocal(
                                rc[:, i, qc : qc + 1], dns[:, i, qc : qc + 1]
                            )
                            nc.scalar.activation(
                                exp2[i][:], exp2[i][:], AF.Copy,
                                scale=rc[:, i, qc : qc + 1],
                            )
                        # transpose blocks into attT
                        for cc in range(qc + 1):
                            for i in range(2):
                                tp = tps.tile([P, P], bf16, tag="tp")
                                nc.tensor.transpose(
                                    tp[:], exp2[i][:, cc * P : (cc + 1) * P], id16[:]
                                )
                                nc.vector.tensor_copy(
                                    attT2[i][:, cc, qc * P : (qc + 1) * P], tp[:]
                                )
                    # att @ V: the two heads write disjoint PE column groups
                    # (output partitions 0-63 / 64-127) -> run concurrent
                    yp = yps.tile([P, L], f32, tag="yp")
                    for cc in range(TC):
                        n0 = cc * P
                        while n0 < L:
                            w = min(512, L - n0)
                            nc.tensor.matmul(
                                yp[0:64, n0 : n0 + w],
                                v_sb[cc][:, h0 * 64 : h0 * 64 + 64],
                                attT2[0][:, cc, n0 : n0 + w],
                                start=(cc == 0),
                                stop=(cc == TC - 1),
                            )
                            nc.tensor.matmul(
                                yp[64:128, n0 : n0 + w],
                                v_sb[cc][:, h1 * 64 : h1 * 64 + 64],
                                attT2[1][:, cc, n0 : n0 + w],
                                start=(cc == 0),
                                stop=(cc == TC - 1),
                                tile_position=(0, 64),
                            )
                            n0 += w
                    nc.vector.tensor_copy(ysb[:, pr, :], yp[:])

            qkv_pool.release()

            # ---------------- proj (partial) + ReduceScatter ----------------
            with tc.tile_pool(name="wproj", bufs=1) as wpp, tc.tile_pool(
                name="asb", bufs=3
            ) as asbp, tc.tile_pool(name="aps", bufs=4, space="PSUM") as apsp:
                wproj_sb = [wpp.tile([P, D], f32r, name=f"wpj{p}") for p in range(4)]
                for p in range(4):
                    nc.sync.dma_start(
                        wproj_sb[p][:], wproj_in[p * P : (p + 1) * P, :].bitcast(f32r)
                    )
                for n in range(2):
                    for t in range(TC):
                        ap_ = apsp.tile([P, 512], f32, tag="ap")
                        for p in range(4):
                            nc.tensor.matmul(
                                ap_[:],
                                ysb[:, p, t * P : (t + 1) * P],
                                wproj_sb[p][:, n * 512 : (n + 1) * 512],
                                start=(p == 0),
                                stop=(p == 3),
                            )
                        asb = asbp.tile([P, 512], f32, tag="asb")
                        nc.scalar.activation(asb[:], ap_[:], AF.Copy)
                        nc.sync.dma_start(
                            cc_in[n][t * P : (t + 1) * P, :], asb[:]
                        )
                    # fire the column-half collective as soon as its inputs
                    # are written; the other half's matmuls overlap it
                    if no_rs:
                        nc.sync.dma_start(cc_out[n][:], cc_in[n][:TMY, :])
                    else:
                        nc.gpsimd.collective_compute(
                            "ReduceScatter",
                            mybir.AluOpType.add,
                            replica_groups=[[0, 1], [2, 3], [4, 5], [6, 7]],
                            ins=[cc_in[n][:]],
                            outs=[cc_out[n][:]],
                        )
            ysb_pool.release()

            # ---------------- residual + LN2 + h2T ----------------
            x2p = es.enter_context(tc.tile_pool(name="x2p", bufs=1))
            x2 = [x2p.tile([P, D], f32, name=f"x2_{t}") for t in range(T2)]
            h2T = [h2Tp.tile([P, TMY], f32r, name=f"h2T{d}") for d in range(DCH)]
            with tc.tile_pool(name="res", bufs=2) as resp, tc.tile_pool(
                name="lnscr2", bufs=2
            ) as lnscr2:
                h2 = [resp.tile([P, D], f32, name=f"h2_{t}", bufs=1) for t in range(T2)]
                for t in range(T2):
                    xm = resp.tile([P, D], f32, tag="xm")
                    ar = resp.tile([P, D], f32, tag="ar")
                    nc.sync.dma_start(xm[:], xmy_in[t * P : (t + 1) * P, :])
                    nc.sync.dma_start(
                        ar[:, 0:512], cc_out[0][t * P : (t + 1) * P, :]
                    )
                    nc.sync.dma_start(
                        ar[:, 512:1024], cc_out[1][t * P : (t + 1) * P, :]
                    )
                    nc.vector.tensor_tensor(x2[t][:], xm[:], ar[:], op=ALU.add)
                    if use_bproj:
                        nc.vector.tensor_tensor(
                            x2[t][:], x2[t][:], bproj_bc[:], op=ALU.add
                        )
                layernorm(
                    tc, nc, (small, lnscr2), x2, h2, T2, aff2, ln2w_bc, ln2b_bc,
                    eps_t[:],
                )
                with tc.tile_pool(name="h2ps", bufs=4, space="PSUM") as h2ps:
                    for t in range(T2):
                        for d in range(DCH):
                            tp = h2ps.tile([P, P], f32, tag="h2p")
                            nc.tensor.transpose(
                                tp[:], h2[t][:, d * P : (d + 1) * P], id32[:]
                            )
                            nc.any.tensor_copy(h2T[d][:, t * P : (t + 1) * P], tp[:])

            # ---------------- FFN ----------------
            m1p = es.enter_context(tc.tile_pool(name="m1p", bufs=1))
            m1T = [m1p.tile([P, TMY], f32r, name=f"m1T{f}") for f in range(FC)]
            with tc.tile_pool(name="wfcp", bufs=6) as wfcp, tc.tile_pool(
                name="fc1ps", bufs=4, space="PSUM"
            ) as fc1ps:
                for fg in range(FC // 4):
                    # one [128, 512] weight tile per d-chunk covers 4 f-chunks
                    # (2KB contiguous rows -> efficient DMA)
                    wts = []
                    for d in range(DCH):
                        if (fg, d) in wfc_pre:
                            wts.append(wfc_pre[(fg, d)])
                            continue
                        wt = wfcp.tile([P, 512], f32r, tag=f"wfc{d % 2}")
                        nc.sync.dma_start(
                            wt[:],
                            wfc_in[d * P : (d + 1) * P, fg * 512 : (fg + 1) * 512]
                            .bitcast(f32r),
                        )
                        wts.append(wt)
                    for fl in range(4):
                        f = fg * 4 + fl
                        mp = fc1ps.tile([P, TMY], f32, tag="m1ps")
                        for d in range(DCH):
                            nc.tensor.matmul(
                                mp[:],
                                wts[d][:, fl * P : (fl + 1) * P],
                                h2T[d][:],
                                start=(d == 0),
                                stop=(d == DCH - 1),
                            )
                        if use_bfc:
                            nc.scalar.activation(
                                m1T[f][:], mp[:], AF.Gelu, bias=bfc_sb[:, f : f + 1]
                            )
                        else:
                            nc.scalar.activation(m1T[f][:], mp[:], AF.Gelu)

            with tc.tile_pool(name="wfc2p", bufs=6) as wfc2p, tc.tile_pool(
                name="outp", bufs=1
            ) as outp, tc.tile_pool(name="fc2ps", bufs=1, space="PSUM") as fc2ps:
                out_sb = [outp.tile([P, D], f32, name=f"o{t}") for t in range(T2)]
                pss = [
                    [fc2ps.tile([P, 512], f32, name=f"fc2_{t}_{n}") for n in range(2)]
                    for t in range(T2)
                ]
                for f in range(FC):
                    for n in range(2):
                        w2 = wfc2p.tile([P, 512], f32r, tag="wfc2")
                        nc.sync.dma_start(
                            w2[:],
                            wfc2_in[f * P : (f + 1) * P, n * 512 : (n + 1) * 512]
                            .bitcast(f32r),
                        )
                        for t in range(T2):
                            nc.tensor.matmul(
                                pss[t][n][:],
                                m1T[f][:, t * P : (t + 1) * P],
                                w2[:],
                                start=(f == 0),
                                stop=(f == FC - 1),
                            )
                for t in range(T2):
                    for n in range(2):
                        nc.vector.tensor_tensor(
                            out_sb[t][:, n * 512 : (n + 1) * 512],
                            pss[t][n][:],
                            x2[t][:, n * 512 : (n + 1) * 512],
                            op=ALU.add,
                        )
                    if use_bfc2:
                        nc.vector.tensor_tensor(
                            out_sb[t][:], out_sb[t][:], bfc2_bc[:], op=ALU.add
                        )
                    nc.sync.dma_start(out_dram[t * P : (t + 1) * P, :], out_sb[t][:])

    nc.compile()
    return nc


def _get_program(flags):
    if flags not in _PROGRAM_CACHE:
        _PROGRAM_CACHE[flags] = _build_program(flags)
    return _PROGRAM_CACHE[flags]


def kernel(
    x,
    ln1_w,
    ln1_b,
    Wqkv,
    bqkv,
    Wproj,
    bproj,
    Er,
    ln2_w,
    ln2_b,
    Wfc,
    bfc,
    Wfc2,
    bfc2,
):
    from concourse.bass_utils import run_bass_kernel_spmd

    x = np.asarray(x, np.float32)
    f = np.float32
    ntriv = lambda a, v: not np.all(np.asarray(a) == v)
    flags = (
        ntriv(ln1_w, 1) or ntriv(ln1_b, 0),
        ntriv(ln2_w, 1) or ntriv(ln2_b, 0),
        ntriv(bqkv[:D], 0),
        ntriv(bqkv[D : 2 * D], 0),
        ntriv(bqkv[2 * D :], 0),
        ntriv(bproj, 0),
        ntriv(bfc, 0),
        ntriv(bfc2, 0),
    )
    nc = _get_program(flags)

    ert2 = np.ascontiguousarray(
        np.concatenate([np.asarray(Er).T, np.asarray(Er).T], axis=0), f
    )
    c = np.ascontiguousarray
    in_maps = []
    for core in range(8):
        b, half = divmod(core, 2)
        hs0, hs1 = half * 512, (half + 1) * 512
        bq = np.asarray(bqkv[:D][hs0:hs1], f) * SCALE
        bk = np.asarray(bqkv[D : 2 * D][hs0:hs1], f)
        in_maps.append(
            {
                "x": c(x[b], f),
                "x_my": c(x[b, hs0:hs1], f),
                "wq": c(np.asarray(Wqkv)[:, 0:D][:, hs0:hs1], f),
                "wk": c(np.asarray(Wqkv)[:, D : 2 * D][:, hs0:hs1], f),
                "wv": c(np.asarray(Wqkv)[:, 2 * D :][:, hs0:hs1], f),
                "wproj": c(np.asarray(Wproj)[hs0:hs1, :], f),
                "ert2": ert2,
                "wfc": c(np.asarray(Wfc), f),
                "wfc2": c(np.asarray(Wfc2), f),
                "ln1a": c(np.asarray(ln1_w), f),
                "ln1b": c(np.asarray(ln1_b), f),
                "ln2a": c(np.asarray(ln2_w), f),
                "ln2b": c(np.asarray(ln2_b), f),
                "bq": c(bq.reshape(4, P).T, f),
                "bk": c(bk.reshape(4, P).T, f),
                "bv": c(np.asarray(bqkv[2 * D :][hs0:hs1]), f),
                "bproj": c(np.asarray(bproj), f),
                "bfc": c(np.asarray(bfc).reshape(FC, P).T, f),
                "bfc2": c(np.asarray(bfc2), f),
            }
        )

    trace = bool(int(os.environ.get("KERNEL_TRACE", "0")))
    res = run_bass_kernel_spmd(nc, in_maps, list(range(8)), trace=trace)
    global LAST_EXEC_NS, LAST_RESULT
    LAST_EXEC_NS = res.exec_time_ns
    LAST_RESULT = res
    out = np.empty((B, L, D), np.float32)
    for core in range(8):
        b, half = divmod(core, 2)
        out[b, half * 512 : (half + 1) * 512] = res.results[core]["out_my"]
    return out


LAST_EXEC_NS = None
LAST_RESULT = None

